# revision 22
# baseline (speedup 1.0000x reference)
"""Trainium2 Bass kernel for nn_C_Cross_Attention3D (B=16, C=768, H=W=64, HEADS=12).

Math (per batch b):
  q   = l2norm_per_head(Wq @ y_b + bq)                      # [12, 64]
  k   = Wk @ x_b + bk                                       # [768, N], N = 4096
  s   = (Qbd^T k) / max(||k||_head, eps)                    # [12, N] cosine scores
  a   = softmax_N(s)                                        # [12, N]
  out = Wp @ (Wv @ (x_b @ a^T |head-diag) + bv) + bp        # [768]

Key restructuring vs. the reference: the V projection commutes with the
attention pooling (one query token per head), so instead of projecting all
N tokens through Wv we pool x with the attention weights first:
  out_attn[head h] = Wv[h_rows, :] @ (x @ a_h^T)  + bv
This halves the dominant GEMM (only K projection runs over all tokens).

Transposes (x^T for the pooling contraction, a^T) are done by DMA-transpose
through a DRAM bounce buffer in bf16, keeping the PE free for matmuls.

Distribution: pure data-parallel over batch, 2 batches per core, 8 cores.
No collectives; host scatters inputs / gathers outputs.

Self-contained: hardcodes all shapes; no sibling imports.
"""

import numpy as np
import ml_dtypes

import concourse.bass as bass
import concourse.mybir as mybir
import concourse.tile as tile
from concourse import bacc
from concourse.bass import ts
from concourse.bass_utils import run_bass_kernel_spmd
from concourse.masks import make_identity

F32 = mybir.dt.float32
BF16 = mybir.dt.bfloat16
AF = mybir.ActivationFunctionType
OP = mybir.AluOpType
AX = mybir.AxisListType

B, C, HEADS, HD = 16, 768, 12, 64
N = 64 * 64                 # tokens per batch
NCORES = 8
BPC = B // NCORES           # batches per core = 2
CT = C // 128               # 6 c-tiles (contraction / channel tiles)
FT = 512                    # token f-tile size
NFT = N // FT               # 8 f-tiles
NNT = N // 128              # 32 n-tiles of 128 tokens
NCH = 4                     # x^T DMA-transpose chunks per batch
CHW = N // NCH              # chunk width in tokens (1024)
EPS = 1e-12


def _act_table_filter():
    """Restrict activation-table choice to the single set that covers all
    funcs this kernel uses (Copy/Exp/Ln/Square), so no mid-kernel
    ACT_TABLE_LOAD swaps are emitted. Index positions are preserved."""
    import functools
    import concourse.bacc as _bacc

    orig = _bacc.get_activation_tables

    @functools.cache
    def filtered(arch):
        t = orig(arch)
        return {
            name: (s if name == "natural_log_exp_and_others" else set())
            for name, s in t.items()
        }

    return orig, filtered


def _build_nc():
    nc = bacc.Bacc(
        "TRN2",
        target_bir_lowering=False,
        debug=False,
        enable_asserts=False,
        num_devices=NCORES,
    )

    x_d = nc.dram_tensor("x", [BPC, C, N], F32, kind="ExternalInput").ap()
    y_d = nc.dram_tensor("y", [C, BPC], F32, kind="ExternalInput").ap()
    wk_d = nc.dram_tensor("wkT", [C, C], BF16, kind="ExternalInput").ap()
    wk2_d = nc.dram_tensor("wk2", [C, C], BF16, kind="ExternalInput").ap()
    wq_d = nc.dram_tensor("wqT", [C, C], BF16, kind="ExternalInput").ap()
    wv_d = nc.dram_tensor("wvT", [C, C], BF16, kind="ExternalInput").ap()
    wp_d = nc.dram_tensor("wpT", [C, C], BF16, kind="ExternalInput").ap()
    bq_d = nc.dram_tensor("bq", [C], F32, kind="ExternalInput").ap()
    bk_d = nc.dram_tensor("bk", [C], F32, kind="ExternalInput").ap()
    bpz_d = nc.dram_tensor("bpz", [C], F32, kind="ExternalInput").ap()
    z_d = nc.dram_tensor("z", [C, BPC], F32, kind="ExternalOutput").ap()

    with tile.TileContext(nc) as tc:
        _emit(nc, tc, x_d, y_d, wk_d, wk2_d, wq_d, wv_d, wp_d, bq_d, bk_d, bpz_d,
              z_d)
    import concourse.bacc as _bacc
    orig, filtered = _act_table_filter()
    _bacc.get_activation_tables = filtered
    try:
        nc.compile()
    finally:
        _bacc.get_activation_tables = orig
    return nc


def _emit(nc, tc, x_d, y_d, wk_d, wk2_d, wq_d, wv_d, wp_d, bq_d, bk_d, bpz_d,
          z_d):
    from contextlib import ExitStack

    ctx = ExitStack()
    with ctx:
        const = ctx.enter_context(tc.tile_pool(name="const", bufs=1))
        statics = ctx.enter_context(tc.tile_pool(name="statics", bufs=1))
        xf_pool = ctx.enter_context(tc.tile_pool(name="xf", bufs=2))
        xb_pool = ctx.enter_context(tc.tile_pool(name="xb", bufs=2))
        k2_pool = ctx.enter_context(tc.tile_pool(name="k2", bufs=2))
        small = ctx.enter_context(tc.tile_pool(name="small", bufs=4))
        at_pool = ctx.enter_context(tc.tile_pool(name="at", bufs=5))
        dram = ctx.enter_context(tc.tile_pool(name="dram", bufs=6, space="DRAM"))
        kp_pool = ctx.enter_context(tc.tile_pool(name="kp", bufs=6, space="PSUM"))
        pp_pool = ctx.enter_context(tc.tile_pool(name="pp", bufs=2, space="PSUM"))

        # ---- constants / weights -------------------------------------------
        # wk on the sync ring first (the K GEMM is the critical path);
        # everything else on the scalar ring so x-tile loads aren't queued
        # behind weights.
        wk_sb = const.tile([128, CT, C], BF16)
        nc.sync.dma_start(wk_sb, wk_d.rearrange("(c p) o -> p c o", p=128))
        wk2_sb = const.tile([128, CT, C], BF16)
        nc.scalar.dma_start(wk2_sb, wk2_d.rearrange("(o p) c -> p o c", p=128))
        wq_sb = const.tile([128, CT, C], BF16)
        nc.scalar.dma_start(wq_sb, wq_d.rearrange("(c p) o -> p c o", p=128))
        bq_sb = const.tile([128, CT], F32)
        nc.scalar.dma_start(bq_sb, bq_d.rearrange("(c p) -> p c", p=128))
        bk_sb = const.tile([128, CT], F32)
        nc.scalar.dma_start(bk_sb, bk_d.rearrange("(c p) -> p c", p=128))
        bpz_sb = const.tile([128, CT], F32)
        nc.scalar.dma_start(bpz_sb, bpz_d.rearrange("(c p) -> p c", p=128))

        id128_f = const.tile([128, 128], F32)
        make_identity(nc, id128_f)
        id64_f = const.tile([64, 64], F32)
        make_identity(nc, id64_f)

        # ones_bd[c, h] = 1 if c // 64 == h  (block-diagonal head indicator)
        ones_bf = const.tile([128, CT, HEADS], BF16)
        ones_f = const.tile([128, CT, HEADS], F32)
        onesT_f = const.tile([HEADS, C], F32)
        nc.vector.memset(ones_bf, 0.0)
        nc.vector.memset(ones_f, 0.0)
        for c in range(CT):
            for half in range(2):
                h = 2 * c + half
                rows = slice(64 * half, 64 * (half + 1))
                nc.vector.memset(ones_bf[rows, c, h : h + 1], 1.0)
                nc.vector.memset(ones_f[rows, c, h : h + 1], 1.0)
        for c in range(CT):
            otp = kp_pool.tile([HEADS, 128], F32, tag="kp")
            nc.tensor.transpose(otp, ones_f[:, c, :], id128_f)
            nc.scalar.copy(out=onesT_f[:, ts(c, 128)], in_=otp)

        # ---- statics --------------------------------------------------------
        scores_b = []
        for b in range(BPC):
            sc_t = statics.tile([44, N], F32, name=f"scores{b}")
            nc.vector.memset(sc_t, 0.0)
            scores_b.append(sc_t)
        xT_all = statics.tile([128, NNT, C], BF16)  # transposed x, current batch
        pooledT_all = statics.tile([64, C], F32)
        nc.vector.memset(pooledT_all, 0.0)

        # ---- q path (both batches at once) ---------------------------------
        y_sb = const.tile([128, CT, BPC], F32)
        nc.scalar.dma_start(y_sb, y_d.rearrange("(c p) b -> p c b", p=128))
        y_bf = const.tile([128, CT, BPC], BF16)
        nc.vector.tensor_copy(out=y_bf, in_=y_sb)

        q_sb = const.tile([128, CT, BPC], F32)
        for o in range(CT):
            qp = kp_pool.tile([128, BPC], F32, tag="kp")
            for c in range(CT):
                nc.tensor.matmul(
                    qp, wq_sb[:, c, ts(o, 128)], y_bf[:, c, :],
                    start=(c == 0), stop=(c == CT - 1),
                )
            nc.vector.tensor_tensor(
                out=q_sb[:, o, :], in0=qp,
                in1=bq_sb[:, o, None].to_broadcast((128, BPC)), op=OP.add,
            )
        q2_sb = const.tile([128, CT, BPC], F32)
        nc.scalar.activation(out=q2_sb, in_=q_sb, func=AF.Square)
        ssqq = kp_pool.tile([HEADS, BPC], F32, tag="kp")
        for c in range(CT):
            nc.tensor.matmul(
                ssqq, ones_f[:, c, :], q2_sb[:, c, :],
                start=(c == 0), stop=(c == CT - 1),
            )
        rq = const.tile([HEADS, BPC], F32)
        nc.scalar.activation(out=rq, in_=ssqq, func=AF.Ln)
        nc.scalar.activation(out=rq, in_=rq, func=AF.Exp, scale=-0.5)
        nc.vector.tensor_scalar_min(rq, rq, 1.0 / EPS)
        # broadcast rq back to channel layout via block-diag ones matmul
        rqbc = kp_pool.tile([128, CT, BPC], F32, tag="kp")
        for c in range(CT):
            nc.tensor.matmul(
                rqbc[:, c, :], onesT_f[:, ts(c, 128)], rq,
                start=(c == 0), stop=(c == CT - 1), skip_group_check=True,
            )
        qn_sb = const.tile([128, CT, BPC], F32)
        nc.vector.tensor_tensor(out=qn_sb, in0=q_sb, in1=rqbc, op=OP.mult)
        # scatter into block-diagonal Qbd [c, 32*b + h]
        qbd_f = const.tile([128, CT, 32 * BPC], F32)
        nc.vector.memset(qbd_f, 0.0)
        for c in range(CT):
            for half in range(2):
                h = 2 * c + half
                rows = slice(64 * half, 64 * (half + 1))
                for b in range(BPC):
                    col = 32 * b + h
                    nc.vector.tensor_copy(
                        out=qbd_f[rows, c, col : col + 1],
                        in_=qn_sb[rows, c, b : b + 1],
                    )
        qbd_bf = const.tile([128, CT, 32 * BPC], BF16)
        nc.vector.tensor_copy(out=qbd_bf, in_=qbd_f)
        # fold q into the K projection: raw scores = (Wk^T Qbd)^T x + Qbd^T bk
        wtld_bf = const.tile([128, CT, 32 * BPC], BF16)
        for m in range(CT):
            wtp = kp_pool.tile([128, 32 * BPC], F32, tag="kp")
            for ot in range(CT):
                nc.tensor.matmul(
                    wtp, wk2_sb[:, ot, ts(m, 128)], qbd_bf[:, ot, :],
                    start=(ot == 0), stop=(ot == CT - 1),
                )
            nc.vector.tensor_copy(out=wtld_bf[:, m, :], in_=wtp)
        qbk_sb = const.tile([32 * BPC, 1], F32)
        qbkp = kp_pool.tile([32 * BPC, 1], F32, tag="kp")
        for ot in range(CT):
            nc.tensor.matmul(
                qbkp, qbd_f[:, ot, :], bk_sb[:, ot, None],
                start=(ot == 0), stop=(ot == CT - 1),
            )
        nc.vector.tensor_copy(out=qbk_sb, in_=qbkp)

        # ---- per-batch phases, software-pipelined ---------------------------
        nmx8_b = [None] * BPC
        attnT_b = [None] * BPC
        rse_b = [None] * BPC

        def pass_a_ftile(b, i, xbd):
            R = slice(32 * b, 32 * b + HEADS)
            scores = scores_b[b]
            x_b = x_d[b].rearrange("(c p) n -> p c n", p=128)
            xf = xf_pool.tile([128, CT, FT], F32, name=f"xf{b}_{i}", tag="xf")
            nc.sync.dma_start(xf, x_b[:, :, ts(i, FT)])
            xb = xb_pool.tile([128, CT, FT], BF16, name=f"xb{b}_{i}", tag="xb")
            nc.vector.tensor_copy(out=xb, in_=xf)
            # write bf16 x to its bounce chunk (2 f-tiles per chunk)
            ch, off = divmod(i * FT, CHW)
            nc.sync.dma_start(
                xbd[ch].rearrange("(c p) n -> p c n", p=128)[:, :, off : off + FT],
                xb,
            )
            if off + FT == CHW:
                # chunk complete -> transposed read into xT_all (scalar ring)
                nc.scalar.dma_start_transpose(
                    xT_all[:, ch * (CHW // 128) : (ch + 1) * (CHW // 128), :],
                    xbd[ch][:],
                )

            k2sb = k2_pool.tile([128, CT, FT], BF16, name=f"k2_{b}_{i}", tag="k2")
            for o in range(CT):
                kp = kp_pool.tile([128, FT], F32, tag="kp")
                for c in range(CT):
                    nc.tensor.matmul(
                        kp, wk_sb[:, c, ts(o, 128)], xb[:, c, :],
                        start=(c == 0), stop=(c == CT - 1),
                    )
                nc.scalar.activation(
                    out=k2sb[:, o, :], in_=kp, func=AF.Square,
                    bias=bk_sb[:, o : o + 1], scale=1.0,
                )

            sp = kp_pool.tile([32 * BPC, FT], F32, tag="kp")
            for c in range(CT):
                nc.tensor.matmul(
                    sp, wtld_bf[:, c, :], xb[:, c, :],
                    start=(c == 0), stop=(c == CT - 1),
                )
            sq = kp_pool.tile([HEADS, FT], F32, tag="kp")
            for c in range(CT):
                nc.tensor.matmul(
                    sq, ones_bf[:, c, :], k2sb[:, c, :],
                    start=(c == 0), stop=(c == CT - 1),
                )
            # r = ssq^-1/2 (clamped to 1/eps); scores = (raw + qbk) * r
            rt = small.tile([HEADS, FT], F32, tag="rt")
            nc.scalar.activation(out=rt, in_=sq, func=AF.Ln)
            nc.scalar.activation(out=rt, in_=rt, func=AF.Exp, scale=-0.5)
            nc.vector.tensor_scalar_min(rt, rt, 1.0 / EPS)
            nc.vector.tensor_scalar(
                out=sp[R, :], in0=sp[R, :],
                scalar1=qbk_sb[R], scalar2=None, op0=OP.add,
            )
            nc.vector.tensor_tensor(
                out=scores[R, ts(i, FT)], in0=sp[R, :], in1=rt, op=OP.mult,
            )
            nc.vector.tensor_reduce(
                nmx8_b[b][R, i : i + 1], scores[R, ts(i, FT)],
                axis=AX.X, op=OP.max)

        def softmax_attn(b):
            R = slice(32 * b, 32 * b + HEADS)
            scores = scores_b[b]
            nmx = small.tile([64, 1], F32, tag="st", name=f"nmx{b}")
            nc.vector.tensor_reduce(
                nmx[R], nmx8_b[b][R, :], axis=AX.X, op=OP.max, negate=True)
            rse = small.tile([64, 1], F32, tag="st", name=f"rse{b}")

            # chunked exp -> SBUF->SBUF transposed read (scalar ring), so
            # pooling can start as soon as the first chunk of attn^T lands.
            attnT_t = []
            se_t = []
            for chk in range(NCH):
                abt = at_pool.tile([64, CHW], BF16, tag="ab", name=f"ab{b}_{chk}")
                sec = small.tile([64, 1], F32, tag="se", name=f"se{b}_{chk}")
                nc.vector.memset(sec[R], 0.0)
                nc.scalar.activation(
                    out=abt[R, :], in_=scores[R, ts(chk, CHW)], func=AF.Exp,
                    bias=nmx[R], scale=1.0, accum_out=sec[R],
                )
                se_t.append(sec)
                att = at_pool.tile(
                    [128, CHW // 128, 32], BF16, tag="attnT", name=f"att{b}_{chk}")
                nc.scalar.dma_start_transpose(
                    att, abt[32 * b : 32 * b + 32, :])
                attnT_t.append(att)
            nc.vector.tensor_tensor(
                out=se_t[0][R], in0=se_t[0][R], in1=se_t[1][R], op=OP.add)
            nc.vector.tensor_tensor(
                out=se_t[2][R], in0=se_t[2][R], in1=se_t[3][R], op=OP.add)
            nc.vector.tensor_tensor(
                out=se_t[0][R], in0=se_t[0][R], in1=se_t[2][R], op=OP.add)
            nc.vector.reciprocal(rse[R], se_t[0][R])
            attnT_b[b] = attnT_t
            rse_b[b] = rse

        def pool(b):
            R = slice(32 * b, 32 * b + HEADS)
            attnT_t = attnT_b[b]
            pp0 = pp_pool.tile([HEADS, 384], F32, tag="pp")
            pp1 = pp_pool.tile([HEADS, 384], F32, tag="pp")
            for nt in range(NNT):
                atl = attnT_t[nt // (CHW // 128)][:, nt % (CHW // 128), 0:HEADS]
                nc.tensor.matmul(
                    pp0, atl, xT_all[:, nt, 0:384],
                    start=(nt == 0), stop=(nt == NNT - 1), skip_group_check=True,
                )
                nc.tensor.matmul(
                    pp1, atl, xT_all[:, nt, 384:768],
                    start=(nt == 0), stop=(nt == NNT - 1), skip_group_check=True,
                )
            nc.vector.tensor_scalar_mul(pooledT_all[R, 0:384], pp0, rse_b[b][R])
            nc.vector.tensor_scalar_mul(pooledT_all[R, 384:768], pp1, rse_b[b][R])

        xbd_b = []
        for b in range(BPC):
            nmx8_b[b] = small.tile([64, NFT], F32, tag="nmx8", name=f"nmx8_{b}")
            xbd_b.append([
                dram.tile([C, CHW], BF16, tag="xbd", name=f"xbd{b}_{t}")
                for t in range(NCH)
            ])

        # batch 0 scores pass + softmax
        for i in range(NFT):
            pass_a_ftile(0, i, xbd_b[0])
        softmax_attn(0)
        # batch 1 first f-tile keeps the PE busy while attn^T(0) lands
        pass_a_ftile(1, 0, xbd_b[1])
        pool(0)
        for i in range(1, NFT):
            pass_a_ftile(1, i, xbd_b[1])
        softmax_attn(1)
        pool(1)

        # ---- tail: out = Wp @ (Wv @ pooled)|diag + bpz ---------------------
        wv_sb = const.tile([128, CT, C], BF16)
        nc.scalar.dma_start(wv_sb, wv_d.rearrange("(c p) o -> p c o", p=128))
        wp_sb = const.tile([128, CT, C], BF16)
        nc.scalar.dma_start(wp_sb, wp_d.rearrange("(c p) o -> p c o", p=128))
        pooled_sb = const.tile([128, CT, BPC * HEADS], BF16)
        for c in range(CT):
            tpp = kp_pool.tile([128, 64], F32, tag="kp")
            nc.tensor.transpose(tpp, pooledT_all[:, ts(c, 128)], id64_f)
            for b in range(BPC):
                nc.vector.tensor_copy(
                    out=pooled_sb[:, c, b * HEADS : (b + 1) * HEADS],
                    in_=tpp[:, 32 * b : 32 * b + HEADS])

        outv_sb = const.tile([128, CT, BPC], BF16)
        for o in range(CT):
            vp = kp_pool.tile([128, BPC * HEADS], F32, tag="kp")
            for c in range(CT):
                nc.tensor.matmul(
                    vp, wv_sb[:, c, ts(o, 128)], pooled_sb[:, c, :],
                    start=(c == 0), stop=(c == CT - 1),
                )
            for half in range(2):
                h = 2 * o + half
                rows = slice(64 * half, 64 * (half + 1))
                for b in range(BPC):
                    col = b * HEADS + h
                    nc.vector.tensor_copy(
                        out=outv_sb[rows, o, b : b + 1],
                        in_=vp[rows, col : col + 1],
                    )

        z_sb = const.tile([128, CT, BPC], F32)
        for o2 in range(CT):
            zp = kp_pool.tile([128, BPC], F32, tag="kp")
            for o in range(CT):
                nc.tensor.matmul(
                    zp, wp_sb[:, o, ts(o2, 128)], outv_sb[:, o, :],
                    start=(o == 0), stop=(o == CT - 1),
                )
            nc.vector.tensor_tensor(
                out=z_sb[:, o2, :], in0=zp,
                in1=bpz_sb[:, o2, None].to_broadcast((128, BPC)), op=OP.add,
            )
        nc.sync.dma_start(z_d.rearrange("(c p) b -> p c b", p=128), z_sb)


_NC_CACHE = None


def _get_nc():
    global _NC_CACHE
    if _NC_CACHE is None:
        _NC_CACHE = _build_nc()
    return _NC_CACHE


def make_in_maps(inputs):
    x = np.ascontiguousarray(np.asarray(inputs["x"], dtype=np.float32)).reshape(B, C, N)
    y = np.asarray(inputs["y"], dtype=np.float32).reshape(B, C)
    Wq = np.asarray(inputs["Wq"], dtype=np.float32)
    bq = np.asarray(inputs["bq"], dtype=np.float32)
    Wkv = np.asarray(inputs["Wkv"], dtype=np.float32)
    bkv = np.asarray(inputs["bkv"], dtype=np.float32)
    Wp = np.asarray(inputs["Wp"], dtype=np.float32)
    bp = np.asarray(inputs["bp"], dtype=np.float32)

    wk, wv = Wkv[:C], Wkv[C:]
    bk, bv = bkv[:C], bkv[C:]
    wkT = np.ascontiguousarray(wk.T).astype(ml_dtypes.bfloat16)
    wk2 = np.ascontiguousarray(wk).astype(ml_dtypes.bfloat16)
    wqT = np.ascontiguousarray(Wq.T).astype(ml_dtypes.bfloat16)
    wvT = np.ascontiguousarray(wv.T).astype(ml_dtypes.bfloat16)
    wpT = np.ascontiguousarray(Wp.T).astype(ml_dtypes.bfloat16)
    bpz = (Wp @ bv + bp).astype(np.float32)

    in_maps = []
    for i in range(NCORES):
        in_maps.append({
            "x": np.ascontiguousarray(x[i * BPC : (i + 1) * BPC]),
            "y": np.ascontiguousarray(y[i * BPC : (i + 1) * BPC].T),
            "wkT": wkT, "wk2": wk2, "wqT": wqT, "wvT": wvT, "wpT": wpT,
            "bq": bq, "bk": np.ascontiguousarray(bk),
            "bpz": bpz,
        })
    return in_maps


def kernel(**inputs):
    nc = _get_nc()
    in_maps = make_in_maps(inputs)
    res = run_bass_kernel_spmd(nc, in_maps, core_ids=list(range(NCORES)))
    z = np.concatenate([r["z"].T for r in res.results], axis=0)
    return z.reshape(B, C, 1, 1).astype(np.float32)


# revision 23
# speedup vs baseline: 1.1276x; 1.1276x over previous
"""Trainium2 Bass kernel for nn_C_Cross_Attention3D (B=16, C=768, H=W=64, HEADS=12).

Math (per batch b):
  q   = l2norm_per_head(Wq @ y_b + bq)                      # [12, 64]
  k   = Wk @ x_b + bk                                       # [768, N], N = 4096
  s   = (Qbd^T k) / max(||k||_head, eps)                    # [12, N] cosine scores
  a   = softmax_N(s)                                        # [12, N]
  out = Wp @ (Wv @ (x_b @ a^T |head-diag) + bv) + bp        # [768]

Key restructuring vs. the reference: the V projection commutes with the
attention pooling (one query token per head), so instead of projecting all
N tokens through Wv we pool x with the attention weights first:
  out_attn[head h] = Wv[h_rows, :] @ (x @ a_h^T)  + bv
This halves the dominant GEMM (only K projection runs over all tokens).

Transposes (x^T for the pooling contraction, a^T) are done by DMA-transpose
through a DRAM bounce buffer in bf16, keeping the PE free for matmuls.

Distribution: pure data-parallel over batch, 2 batches per core, 8 cores.
No collectives; host scatters inputs / gathers outputs.

Self-contained: hardcodes all shapes; no sibling imports.
"""

import numpy as np
import ml_dtypes

import concourse.bass as bass
import concourse.mybir as mybir
import concourse.tile as tile
from concourse import bacc
from concourse.bass import ts
from concourse.bass_utils import run_bass_kernel_spmd
from concourse.masks import make_identity

F32 = mybir.dt.float32
BF16 = mybir.dt.bfloat16
AF = mybir.ActivationFunctionType
OP = mybir.AluOpType
AX = mybir.AxisListType

B, C, HEADS, HD = 16, 768, 12, 64
N = 64 * 64                 # tokens per batch
NCORES = 8
BPC = B // NCORES           # batches per core = 2
CT = C // 128               # 6 c-tiles (contraction / channel tiles)
FT = 512                    # token f-tile size
NFT = N // FT               # 8 f-tiles
NNT = N // 128              # 32 n-tiles of 128 tokens
NCH = 4                     # x^T DMA-transpose chunks per batch
CHW = N // NCH              # chunk width in tokens (1024)
EPS = 1e-12


def _act_table_filter():
    """Restrict activation-table choice to the single set that covers all
    funcs this kernel uses (Copy/Exp/Ln/Square), so no mid-kernel
    ACT_TABLE_LOAD swaps are emitted. Index positions are preserved."""
    import functools
    import concourse.bacc as _bacc

    orig = _bacc.get_activation_tables

    @functools.cache
    def filtered(arch):
        t = orig(arch)
        return {
            name: (s if name == "natural_log_exp_and_others" else set())
            for name, s in t.items()
        }

    return orig, filtered


def _build_nc():
    nc = bacc.Bacc(
        "TRN2",
        target_bir_lowering=False,
        debug=False,
        enable_asserts=False,
        num_devices=NCORES,
    )

    x_d = nc.dram_tensor("x", [BPC, C, N], F32, kind="ExternalInput").ap()
    y_d = nc.dram_tensor("y", [C, BPC], F32, kind="ExternalInput").ap()
    wk_d = nc.dram_tensor("wkT", [C, C], BF16, kind="ExternalInput").ap()
    wk2_d = nc.dram_tensor("wk2", [C, C], BF16, kind="ExternalInput").ap()
    wq_d = nc.dram_tensor("wqT", [C, C], BF16, kind="ExternalInput").ap()
    wv_d = nc.dram_tensor("wvT", [C, C], BF16, kind="ExternalInput").ap()
    wp_d = nc.dram_tensor("wpT", [C, C], BF16, kind="ExternalInput").ap()
    bq_d = nc.dram_tensor("bq", [C], F32, kind="ExternalInput").ap()
    bk_d = nc.dram_tensor("bk", [C], F32, kind="ExternalInput").ap()
    bpz_d = nc.dram_tensor("bpz", [C], F32, kind="ExternalInput").ap()
    z_d = nc.dram_tensor("z", [C, BPC], F32, kind="ExternalOutput").ap()

    with tile.TileContext(nc) as tc:
        _emit(nc, tc, x_d, y_d, wk_d, wk2_d, wq_d, wv_d, wp_d, bq_d, bk_d, bpz_d,
              z_d)
    import concourse.bacc as _bacc
    orig, filtered = _act_table_filter()
    _bacc.get_activation_tables = filtered
    try:
        nc.compile()
    finally:
        _bacc.get_activation_tables = orig
    return nc


def _emit(nc, tc, x_d, y_d, wk_d, wk2_d, wq_d, wv_d, wp_d, bq_d, bk_d, bpz_d,
          z_d):
    from contextlib import ExitStack

    ctx = ExitStack()
    with ctx:
        const = ctx.enter_context(tc.tile_pool(name="const", bufs=1))
        statics = ctx.enter_context(tc.tile_pool(name="statics", bufs=1))
        xf_pool = ctx.enter_context(tc.tile_pool(name="xf", bufs=2))
        xb_pool = ctx.enter_context(tc.tile_pool(name="xb", bufs=3))
        k2_pool = ctx.enter_context(tc.tile_pool(name="k2", bufs=3))
        small = ctx.enter_context(tc.tile_pool(name="small", bufs=4))
        at_pool = ctx.enter_context(tc.tile_pool(name="at", bufs=5))
        dram = ctx.enter_context(tc.tile_pool(name="dram", bufs=6, space="DRAM"))
        kp_pool = ctx.enter_context(tc.tile_pool(name="kp", bufs=6, space="PSUM"))
        pp_pool = ctx.enter_context(tc.tile_pool(name="pp", bufs=2, space="PSUM"))

        # ---- weights: wk first on the sync ring (K GEMM is critical path) --
        wk_sb = const.tile([128, CT, C], BF16)
        nc.sync.dma_start(wk_sb, wk_d.rearrange("(c p) o -> p c o", p=128))
        wq_sb = const.tile([128, CT, C], BF16)
        nc.sync.dma_start(wq_sb, wq_d.rearrange("(c p) o -> p c o", p=128))
        wk2_sb = const.tile([128, CT, C], BF16)
        nc.sync.dma_start(wk2_sb, wk2_d.rearrange("(o p) c -> p o c", p=128))
        bq_sb = const.tile([128, CT], F32)
        nc.scalar.dma_start(bq_sb, bq_d.rearrange("(c p) -> p c", p=128))
        bk_sb = const.tile([128, CT], F32)
        nc.scalar.dma_start(bk_sb, bk_d.rearrange("(c p) -> p c", p=128))
        bpz_sb = const.tile([128, CT], F32)
        nc.scalar.dma_start(bpz_sb, bpz_d.rearrange("(c p) -> p c", p=128))
        y_sb = const.tile([128, CT, BPC], F32)
        nc.scalar.dma_start(y_sb, y_d.rearrange("(c p) b -> p c b", p=128))

        id128_f = const.tile([128, 128], F32)
        make_identity(nc, id128_f)
        id64_f = const.tile([64, 64], F32)
        make_identity(nc, id64_f)

        # ones_bd[c, h] = 1 if c // 64 == h  (block-diagonal head indicator)
        ones_bf = const.tile([128, CT, HEADS], BF16)
        ones_f = const.tile([128, CT, HEADS], F32)
        onesT_f = const.tile([HEADS, C], F32)
        nc.vector.memset(ones_bf, 0.0)
        nc.vector.memset(ones_f, 0.0)
        for c in range(CT):
            for half in range(2):
                h = 2 * c + half
                rows = slice(64 * half, 64 * (half + 1))
                nc.vector.memset(ones_bf[rows, c, h : h + 1], 1.0)
                nc.vector.memset(ones_f[rows, c, h : h + 1], 1.0)

        # ---- statics --------------------------------------------------------
        scores_all = statics.tile([64, N], F32)   # rows: 32*b + h
        nc.vector.memset(scores_all, 0.0)
        xT_all = statics.tile([128, NNT, C], BF16)  # transposed x, current batch
        pooledT_all = statics.tile([64, C], F32)
        nc.vector.memset(pooledT_all, 0.0)

        wtld_bf = const.tile([128, CT, 32 * BPC], BF16)
        qbk_sb = const.tile([32 * BPC, 1], F32)

        def qpath():
            y_bf = const.tile([128, CT, BPC], BF16)
            nc.vector.tensor_copy(out=y_bf, in_=y_sb)
            for c in range(CT):
                otp = kp_pool.tile([HEADS, 128], F32, tag="kp")
                nc.tensor.transpose(otp, ones_f[:, c, :], id128_f)
                nc.scalar.copy(out=onesT_f[:, ts(c, 128)], in_=otp)
            q_sb = const.tile([128, CT, BPC], F32)
            for o in range(CT):
                qp = kp_pool.tile([128, BPC], F32, tag="kp")
                for c in range(CT):
                    nc.tensor.matmul(
                        qp, wq_sb[:, c, ts(o, 128)], y_bf[:, c, :],
                        start=(c == 0), stop=(c == CT - 1),
                    )
                nc.vector.tensor_tensor(
                    out=q_sb[:, o, :], in0=qp,
                    in1=bq_sb[:, o, None].to_broadcast((128, BPC)), op=OP.add,
                )
            q2_sb = const.tile([128, CT, BPC], F32)
            nc.scalar.activation(out=q2_sb, in_=q_sb, func=AF.Square)
            ssqq = kp_pool.tile([HEADS, BPC], F32, tag="kp")
            for c in range(CT):
                nc.tensor.matmul(
                    ssqq, ones_f[:, c, :], q2_sb[:, c, :],
                    start=(c == 0), stop=(c == CT - 1),
                )
            rq = const.tile([HEADS, BPC], F32)
            nc.scalar.activation(out=rq, in_=ssqq, func=AF.Ln)
            nc.scalar.activation(out=rq, in_=rq, func=AF.Exp, scale=-0.5)
            nc.vector.tensor_scalar_min(rq, rq, 1.0 / EPS)
            rqbc = kp_pool.tile([128, CT, BPC], F32, tag="kp")
            for c in range(CT):
                nc.tensor.matmul(
                    rqbc[:, c, :], onesT_f[:, ts(c, 128)], rq,
                    start=(c == 0), stop=(c == CT - 1), skip_group_check=True,
                )
            qn_sb = const.tile([128, CT, BPC], F32)
            nc.vector.tensor_tensor(out=qn_sb, in0=q_sb, in1=rqbc, op=OP.mult)
            qbd_f = const.tile([128, CT, 32 * BPC], F32)
            nc.vector.memset(qbd_f, 0.0)
            for c in range(CT):
                for half in range(2):
                    h = 2 * c + half
                    rows = slice(64 * half, 64 * (half + 1))
                    for b in range(BPC):
                        col = 32 * b + h
                        nc.vector.tensor_copy(
                            out=qbd_f[rows, c, col : col + 1],
                            in_=qn_sb[rows, c, b : b + 1],
                        )
            qbd_bf = const.tile([128, CT, 32 * BPC], BF16)
            nc.vector.tensor_copy(out=qbd_bf, in_=qbd_f)
            # fold q into the K projection: raw = (Wk^T Qbd)^T x + Qbd^T bk
            for m in range(CT):
                wtp = kp_pool.tile([128, 32 * BPC], F32, tag="kp")
                for ot in range(CT):
                    nc.tensor.matmul(
                        wtp, wk2_sb[:, ot, ts(m, 128)], qbd_bf[:, ot, :],
                        start=(ot == 0), stop=(ot == CT - 1),
                    )
                nc.vector.tensor_copy(out=wtld_bf[:, m, :], in_=wtp)
            qbkp = kp_pool.tile([32 * BPC, 1], F32, tag="kp")
            for ot in range(CT):
                nc.tensor.matmul(
                    qbkp, qbd_f[:, ot, :], bk_sb[:, ot, None],
                    start=(ot == 0), stop=(ot == CT - 1),
                )
            nc.vector.tensor_copy(out=qbk_sb, in_=qbkp)

        # ---- per-batch pass A, split into k-part / score-part ---------------
        nmx8_b = [None] * BPC
        attnT_b = [None] * BPC
        rse_b = [None] * BPC
        xb_t = {}
        k2_t = {}
        pending_tp = {0: [], 1: []}

        def kpart(b, i, xbd, defer_tp=False):
            x_b = x_d[b].rearrange("(c p) n -> p c n", p=128)
            xf = xf_pool.tile([128, CT, FT], F32, name=f"xf{b}_{i}", tag="xf")
            nc.sync.dma_start(xf, x_b[:, :, ts(i, FT)])
            xb = xb_pool.tile([128, CT, FT], BF16, name=f"xb{b}_{i}", tag="xb")
            nc.vector.tensor_copy(out=xb, in_=xf)
            xb_t[(b, i)] = xb
            # bf16 bounce (gpsimd/SWDGE ring), 2 f-tiles per chunk
            ch, off = divmod(i * FT, CHW)
            nc.gpsimd.dma_start(
                xbd[ch].rearrange("(c p) n -> p c n", p=128)[:, :, off : off + FT],
                xb,
            )
            if off + FT == CHW:
                if defer_tp:
                    pending_tp[b].append(ch)
                else:
                    nc.sync.dma_start_transpose(
                        xT_all[:, ch * (CHW // 128) : (ch + 1) * (CHW // 128), :],
                        xbd[ch][:],
                    )
            k2sb = k2_pool.tile([128, CT, FT], BF16, name=f"k2_{b}_{i}", tag="k2")
            k2_t[(b, i)] = k2sb
            for o in range(CT):
                kp = kp_pool.tile([128, FT], F32, tag="kp")
                for c in range(CT):
                    nc.tensor.matmul(
                        kp, wk_sb[:, c, ts(o, 128)], xb[:, c, :],
                        start=(c == 0), stop=(c == CT - 1),
                    )
                nc.scalar.activation(
                    out=k2sb[:, o, :], in_=kp, func=AF.Square,
                    bias=bk_sb[:, o : o + 1], scale=1.0,
                )

        def flush_tp(b, xbd):
            for ch in pending_tp[b]:
                nc.sync.dma_start_transpose(
                    xT_all[:, ch * (CHW // 128) : (ch + 1) * (CHW // 128), :],
                    xbd[ch][:],
                )
            pending_tp[b] = []

        def spart(b, i):
            R = slice(32 * b, 32 * b + HEADS)
            xb = xb_t.pop((b, i))
            k2sb = k2_t.pop((b, i))
            sp = kp_pool.tile([32 * BPC, FT], F32, tag="kp")
            for c in range(CT):
                nc.tensor.matmul(
                    sp, wtld_bf[:, c, :], xb[:, c, :],
                    start=(c == 0), stop=(c == CT - 1),
                )
            sq = kp_pool.tile([HEADS, FT], F32, tag="kp")
            for c in range(CT):
                nc.tensor.matmul(
                    sq, ones_bf[:, c, :], k2sb[:, c, :],
                    start=(c == 0), stop=(c == CT - 1),
                )
            rt = small.tile([HEADS, FT], F32, tag="rt")
            nc.scalar.activation(out=rt, in_=sq, func=AF.Ln)
            nc.scalar.activation(out=rt, in_=rt, func=AF.Exp, scale=-0.5)
            nc.vector.tensor_scalar_min(rt, rt, 1.0 / EPS)
            nc.vector.tensor_scalar(
                out=sp[R, :], in0=sp[R, :],
                scalar1=qbk_sb[R], scalar2=None, op0=OP.add,
            )
            nc.vector.tensor_tensor(
                out=scores_all[R, ts(i, FT)], in0=sp[R, :], in1=rt, op=OP.mult,
            )
            nc.vector.tensor_reduce(
                nmx8_b[b][R, i : i + 1], scores_all[R, ts(i, FT)],
                axis=AX.X, op=OP.max)

        def softmax_attn(b):
            R = slice(32 * b, 32 * b + HEADS)
            nmx = small.tile([64, 1], F32, tag="st", name=f"nmx{b}")
            nc.vector.tensor_reduce(
                nmx[R], nmx8_b[b][R, :], axis=AX.X, op=OP.max, negate=True)
            rse = small.tile([64, 1], F32, tag="st", name=f"rse{b}")
            attnT_t = []
            se_t = []
            for chk in range(NCH):
                abt = at_pool.tile([64, CHW], BF16, tag="ab", name=f"ab{b}_{chk}")
                sec = small.tile([64, 1], F32, tag="se", name=f"se{b}_{chk}")
                nc.vector.memset(sec[R], 0.0)
                nc.scalar.activation(
                    out=abt[R, :], in_=scores_all[R, ts(chk, CHW)], func=AF.Exp,
                    bias=nmx[R], scale=1.0, accum_out=sec[R],
                )
                se_t.append(sec)
                att = at_pool.tile(
                    [128, CHW // 128, 32], BF16, tag="attnT", name=f"att{b}_{chk}")
                nc.sync.dma_start_transpose(att, abt[32 * b : 32 * b + 32, :])
                attnT_t.append(att)
            nc.vector.tensor_tensor(
                out=se_t[0][R], in0=se_t[0][R], in1=se_t[1][R], op=OP.add)
            nc.vector.tensor_tensor(
                out=se_t[2][R], in0=se_t[2][R], in1=se_t[3][R], op=OP.add)
            nc.vector.tensor_tensor(
                out=se_t[0][R], in0=se_t[0][R], in1=se_t[2][R], op=OP.add)
            nc.vector.reciprocal(rse[R], se_t[0][R])
            attnT_b[b] = attnT_t
            rse_b[b] = rse

        def pool(b):
            R = slice(32 * b, 32 * b + HEADS)
            attnT_t = attnT_b[b]
            pp0 = pp_pool.tile([HEADS, 384], F32, tag="pp")
            pp1 = pp_pool.tile([HEADS, 384], F32, tag="pp")
            for nt in range(NNT):
                atl = attnT_t[nt // (CHW // 128)][:, nt % (CHW // 128), 0:HEADS]
                nc.tensor.matmul(
                    pp0, atl, xT_all[:, nt, 0:384],
                    start=(nt == 0), stop=(nt == NNT - 1), skip_group_check=True,
                )
                nc.tensor.matmul(
                    pp1, atl, xT_all[:, nt, 384:768],
                    start=(nt == 0), stop=(nt == NNT - 1), skip_group_check=True,
                )
            nc.vector.tensor_scalar_mul(pooledT_all[R, 0:384], pp0, rse_b[b][R])
            nc.vector.tensor_scalar_mul(pooledT_all[R, 384:768], pp1, rse_b[b][R])

        xbd_b = []
        for b in range(BPC):
            nmx8_b[b] = small.tile([64, NFT], F32, tag="nmx8", name=f"nmx8_{b}")
            xbd_b.append([
                dram.tile([C, CHW], BF16, tag="xbd", name=f"xbd{b}_{t}")
                for t in range(NCH)
            ])

        # ---- schedule -------------------------------------------------------
        kpart(0, 0, xbd_b[0])
        kpart(0, 1, xbd_b[0])
        qpath()
        for i in range(NFT):
            if i + 2 < NFT:
                kpart(0, i + 2, xbd_b[0])
            spart(0, i)
        softmax_attn(0)
        kpart(1, 0, xbd_b[1], defer_tp=True)
        kpart(1, 1, xbd_b[1], defer_tp=True)
        pool(0)
        flush_tp(1, xbd_b[1])
        for i in range(NFT):
            if i + 2 < NFT:
                kpart(1, i + 2, xbd_b[1])
            spart(1, i)
        softmax_attn(1)
        pool(1)

        # ---- tail: out = Wp @ (Wv @ pooled)|diag + bpz ---------------------
        wv_sb = const.tile([128, CT, C], BF16)
        nc.scalar.dma_start(wv_sb, wv_d.rearrange("(c p) o -> p c o", p=128))
        wp_sb = const.tile([128, CT, C], BF16)
        nc.scalar.dma_start(wp_sb, wp_d.rearrange("(c p) o -> p c o", p=128))
        pooled_sb = const.tile([128, CT, BPC * HEADS], BF16)
        for c in range(CT):
            tpp = kp_pool.tile([128, 64], F32, tag="kp")
            nc.tensor.transpose(tpp, pooledT_all[:, ts(c, 128)], id64_f)
            for b in range(BPC):
                nc.vector.tensor_copy(
                    out=pooled_sb[:, c, b * HEADS : (b + 1) * HEADS],
                    in_=tpp[:, 32 * b : 32 * b + HEADS])

        outv_sb = const.tile([128, CT, BPC], BF16)
        for o in range(CT):
            vp = kp_pool.tile([128, BPC * HEADS], F32, tag="kp")
            for c in range(CT):
                nc.tensor.matmul(
                    vp, wv_sb[:, c, ts(o, 128)], pooled_sb[:, c, :],
                    start=(c == 0), stop=(c == CT - 1),
                )
            for half in range(2):
                h = 2 * o + half
                rows = slice(64 * half, 64 * (half + 1))
                for b in range(BPC):
                    col = b * HEADS + h
                    nc.vector.tensor_copy(
                        out=outv_sb[rows, o, b : b + 1],
                        in_=vp[rows, col : col + 1],
                    )

        z_sb = const.tile([128, CT, BPC], F32)
        for o2 in range(CT):
            zp = kp_pool.tile([128, BPC], F32, tag="kp")
            for o in range(CT):
                nc.tensor.matmul(
                    zp, wp_sb[:, o, ts(o2, 128)], outv_sb[:, o, :],
                    start=(o == 0), stop=(o == CT - 1),
                )
            nc.vector.tensor_tensor(
                out=z_sb[:, o2, :], in0=zp,
                in1=bpz_sb[:, o2, None].to_broadcast((128, BPC)), op=OP.add,
            )
        nc.sync.dma_start(z_d.rearrange("(c p) b -> p c b", p=128), z_sb)


_NC_CACHE = None


def _get_nc():
    global _NC_CACHE
    if _NC_CACHE is None:
        _NC_CACHE = _build_nc()
    return _NC_CACHE


def make_in_maps(inputs):
    x = np.ascontiguousarray(np.asarray(inputs["x"], dtype=np.float32)).reshape(B, C, N)
    y = np.asarray(inputs["y"], dtype=np.float32).reshape(B, C)
    Wq = np.asarray(inputs["Wq"], dtype=np.float32)
    bq = np.asarray(inputs["bq"], dtype=np.float32)
    Wkv = np.asarray(inputs["Wkv"], dtype=np.float32)
    bkv = np.asarray(inputs["bkv"], dtype=np.float32)
    Wp = np.asarray(inputs["Wp"], dtype=np.float32)
    bp = np.asarray(inputs["bp"], dtype=np.float32)

    wk, wv = Wkv[:C], Wkv[C:]
    bk, bv = bkv[:C], bkv[C:]
    wkT = np.ascontiguousarray(wk.T).astype(ml_dtypes.bfloat16)
    wk2 = np.ascontiguousarray(wk).astype(ml_dtypes.bfloat16)
    wqT = np.ascontiguousarray(Wq.T).astype(ml_dtypes.bfloat16)
    wvT = np.ascontiguousarray(wv.T).astype(ml_dtypes.bfloat16)
    wpT = np.ascontiguousarray(Wp.T).astype(ml_dtypes.bfloat16)
    bpz = (Wp @ bv + bp).astype(np.float32)

    in_maps = []
    for i in range(NCORES):
        in_maps.append({
            "x": np.ascontiguousarray(x[i * BPC : (i + 1) * BPC]),
            "y": np.ascontiguousarray(y[i * BPC : (i + 1) * BPC].T),
            "wkT": wkT, "wk2": wk2, "wqT": wqT, "wvT": wvT, "wpT": wpT,
            "bq": bq, "bk": np.ascontiguousarray(bk),
            "bpz": bpz,
        })
    return in_maps


def kernel(**inputs):
    nc = _get_nc()
    in_maps = make_in_maps(inputs)
    res = run_bass_kernel_spmd(nc, in_maps, core_ids=list(range(NCORES)))
    z = np.concatenate([r["z"].T for r in res.results], axis=0)
    return z.reshape(B, C, 1, 1).astype(np.float32)


# revision 24
# speedup vs baseline: 1.1350x; 1.0066x over previous
"""Trainium2 Bass kernel for nn_C_Cross_Attention3D (B=16, C=768, H=W=64, HEADS=12).

Math (per batch b):
  q   = l2norm_per_head(Wq @ y_b + bq)                      # [12, 64]
  k   = Wk @ x_b + bk                                       # [768, N], N = 4096
  s   = (Qbd^T k) / max(||k||_head, eps)                    # [12, N] cosine scores
  a   = softmax_N(s)                                        # [12, N]
  out = Wp @ (Wv @ (x_b @ a^T |head-diag) + bv) + bp        # [768]

Key restructuring vs. the reference: the V projection commutes with the
attention pooling (one query token per head), so instead of projecting all
N tokens through Wv we pool x with the attention weights first:
  out_attn[head h] = Wv[h_rows, :] @ (x @ a_h^T)  + bv
This halves the dominant GEMM (only K projection runs over all tokens).

Transposes (x^T for the pooling contraction, a^T) are done by DMA-transpose
through a DRAM bounce buffer in bf16, keeping the PE free for matmuls.

Distribution: pure data-parallel over batch, 2 batches per core, 8 cores.
No collectives; host scatters inputs / gathers outputs.

Self-contained: hardcodes all shapes; no sibling imports.
"""

import numpy as np
import ml_dtypes

import concourse.bass as bass
import concourse.mybir as mybir
import concourse.tile as tile
from concourse import bacc
from concourse.bass import ts
from concourse.bass_utils import run_bass_kernel_spmd
from concourse.masks import make_identity

F32 = mybir.dt.float32
BF16 = mybir.dt.bfloat16
AF = mybir.ActivationFunctionType
OP = mybir.AluOpType
AX = mybir.AxisListType

B, C, HEADS, HD = 16, 768, 12, 64
N = 64 * 64                 # tokens per batch
NCORES = 8
BPC = B // NCORES           # batches per core = 2
CT = C // 128               # 6 c-tiles (contraction / channel tiles)
FT = 512                    # token f-tile size
NFT = N // FT               # 8 f-tiles
NNT = N // 128              # 32 n-tiles of 128 tokens
NCH = 4                     # x^T DMA-transpose chunks per batch
CHW = N // NCH              # chunk width in tokens (1024)
EPS = 1e-12


def _act_table_filter():
    """Restrict activation-table choice to the single set that covers all
    funcs this kernel uses (Copy/Exp/Ln/Square), so no mid-kernel
    ACT_TABLE_LOAD swaps are emitted. Index positions are preserved."""
    import functools
    import concourse.bacc as _bacc

    orig = _bacc.get_activation_tables

    @functools.cache
    def filtered(arch):
        t = orig(arch)
        return {
            name: (s if name == "natural_log_exp_and_others" else set())
            for name, s in t.items()
        }

    return orig, filtered


def _build_nc():
    nc = bacc.Bacc(
        "TRN2",
        target_bir_lowering=False,
        debug=False,
        enable_asserts=False,
        num_devices=NCORES,
    )

    x_d = nc.dram_tensor("x", [BPC, C, N], F32, kind="ExternalInput").ap()
    y_d = nc.dram_tensor("y", [C, BPC], F32, kind="ExternalInput").ap()
    wk_d = nc.dram_tensor("wkT", [C, C], BF16, kind="ExternalInput").ap()
    wk2_d = nc.dram_tensor("wk2", [C, C], BF16, kind="ExternalInput").ap()
    wq_d = nc.dram_tensor("wqT", [C, C], BF16, kind="ExternalInput").ap()
    wv_d = nc.dram_tensor("wvT", [C, C], BF16, kind="ExternalInput").ap()
    wp_d = nc.dram_tensor("wpT", [C, C], BF16, kind="ExternalInput").ap()
    bq_d = nc.dram_tensor("bq", [C], F32, kind="ExternalInput").ap()
    bk_d = nc.dram_tensor("bk", [C], F32, kind="ExternalInput").ap()
    bpz_d = nc.dram_tensor("bpz", [C], F32, kind="ExternalInput").ap()
    z_d = nc.dram_tensor("z", [C, BPC], F32, kind="ExternalOutput").ap()

    with tile.TileContext(nc) as tc:
        _emit(nc, tc, x_d, y_d, wk_d, wk2_d, wq_d, wv_d, wp_d, bq_d, bk_d, bpz_d,
              z_d)
    import concourse.bacc as _bacc
    orig, filtered = _act_table_filter()
    _bacc.get_activation_tables = filtered
    try:
        nc.compile()
    finally:
        _bacc.get_activation_tables = orig
    return nc


def _emit(nc, tc, x_d, y_d, wk_d, wk2_d, wq_d, wv_d, wp_d, bq_d, bk_d, bpz_d,
          z_d):
    from contextlib import ExitStack

    ctx = ExitStack()
    with ctx:
        const = ctx.enter_context(tc.tile_pool(name="const", bufs=1))
        statics = ctx.enter_context(tc.tile_pool(name="statics", bufs=1))
        xf_pool = ctx.enter_context(tc.tile_pool(name="xf", bufs=2))
        xb_pool = ctx.enter_context(tc.tile_pool(name="xb", bufs=3))
        k2_pool = ctx.enter_context(tc.tile_pool(name="k2", bufs=3))
        small = ctx.enter_context(tc.tile_pool(name="small", bufs=4))
        at_pool = ctx.enter_context(tc.tile_pool(name="at", bufs=5))
        dram = ctx.enter_context(tc.tile_pool(name="dram", bufs=6, space="DRAM"))
        kp_pool = ctx.enter_context(tc.tile_pool(name="kp", bufs=6, space="PSUM"))
        pp_pool = ctx.enter_context(tc.tile_pool(name="pp", bufs=2, space="PSUM"))

        # ---- weights: wk first on the sync ring (K GEMM is critical path) --
        wk_sb = const.tile([128, CT, C], BF16)
        nc.sync.dma_start(wk_sb, wk_d.rearrange("(c p) o -> p c o", p=128))
        wq_sb = const.tile([128, CT, C], BF16)
        nc.scalar.dma_start(wq_sb, wq_d.rearrange("(c p) o -> p c o", p=128))
        wk2_sb = const.tile([128, CT, C], BF16)
        nc.scalar.dma_start(wk2_sb, wk2_d.rearrange("(o p) c -> p o c", p=128))
        bq_sb = const.tile([128, CT], F32)
        nc.scalar.dma_start(bq_sb, bq_d.rearrange("(c p) -> p c", p=128))
        bk_sb = const.tile([128, CT], F32)
        nc.scalar.dma_start(bk_sb, bk_d.rearrange("(c p) -> p c", p=128))
        bpz_sb = const.tile([128, CT], F32)
        nc.scalar.dma_start(bpz_sb, bpz_d.rearrange("(c p) -> p c", p=128))
        y_sb = const.tile([128, CT, BPC], F32)
        nc.scalar.dma_start(y_sb, y_d.rearrange("(c p) b -> p c b", p=128))

        id128_f = const.tile([128, 128], F32)
        make_identity(nc, id128_f)
        id64_f = const.tile([64, 64], F32)
        make_identity(nc, id64_f)

        # ones_bd[c, h] = 1 if c // 64 == h  (block-diagonal head indicator)
        ones_bf = const.tile([128, CT, HEADS], BF16)
        ones_f = const.tile([128, CT, HEADS], F32)
        onesT_f = const.tile([HEADS, C], F32)
        nc.vector.memset(ones_bf, 0.0)
        nc.vector.memset(ones_f, 0.0)
        for c in range(CT):
            for half in range(2):
                h = 2 * c + half
                rows = slice(64 * half, 64 * (half + 1))
                nc.vector.memset(ones_bf[rows, c, h : h + 1], 1.0)
                nc.vector.memset(ones_f[rows, c, h : h + 1], 1.0)

        # ---- statics --------------------------------------------------------
        scores_all = statics.tile([64, N], F32)   # rows: 32*b + h
        nc.vector.memset(scores_all, 0.0)
        xT_all = statics.tile([128, NNT, C], BF16)  # transposed x, current batch
        pooledT_all = statics.tile([64, C], F32)
        nc.vector.memset(pooledT_all, 0.0)

        wtld_bf = const.tile([128, CT, 32 * BPC], BF16)
        qbk_sb = const.tile([32 * BPC, 1], F32)

        def qpath():
            y_bf = const.tile([128, CT, BPC], BF16)
            nc.vector.tensor_copy(out=y_bf, in_=y_sb)
            for c in range(CT):
                otp = kp_pool.tile([HEADS, 128], F32, tag="kp")
                nc.tensor.transpose(otp, ones_f[:, c, :], id128_f)
                nc.scalar.copy(out=onesT_f[:, ts(c, 128)], in_=otp)
            q_sb = const.tile([128, CT, BPC], F32)
            for o in range(CT):
                qp = kp_pool.tile([128, BPC], F32, tag="kp")
                for c in range(CT):
                    nc.tensor.matmul(
                        qp, wq_sb[:, c, ts(o, 128)], y_bf[:, c, :],
                        start=(c == 0), stop=(c == CT - 1),
                    )
                nc.vector.tensor_tensor(
                    out=q_sb[:, o, :], in0=qp,
                    in1=bq_sb[:, o, None].to_broadcast((128, BPC)), op=OP.add,
                )
            q2_sb = const.tile([128, CT, BPC], F32)
            nc.scalar.activation(out=q2_sb, in_=q_sb, func=AF.Square)
            ssqq = kp_pool.tile([HEADS, BPC], F32, tag="kp")
            for c in range(CT):
                nc.tensor.matmul(
                    ssqq, ones_f[:, c, :], q2_sb[:, c, :],
                    start=(c == 0), stop=(c == CT - 1),
                )
            rq = const.tile([HEADS, BPC], F32)
            nc.scalar.activation(out=rq, in_=ssqq, func=AF.Ln)
            nc.scalar.activation(out=rq, in_=rq, func=AF.Exp, scale=-0.5)
            nc.vector.tensor_scalar_min(rq, rq, 1.0 / EPS)
            rqbc = kp_pool.tile([128, CT, BPC], F32, tag="kp")
            for c in range(CT):
                nc.tensor.matmul(
                    rqbc[:, c, :], onesT_f[:, ts(c, 128)], rq,
                    start=(c == 0), stop=(c == CT - 1), skip_group_check=True,
                )
            qn_sb = const.tile([128, CT, BPC], F32)
            nc.vector.tensor_tensor(out=qn_sb, in0=q_sb, in1=rqbc, op=OP.mult)
            qbd_f = const.tile([128, CT, 32 * BPC], F32)
            nc.vector.memset(qbd_f, 0.0)
            for c in range(CT):
                for half in range(2):
                    h = 2 * c + half
                    rows = slice(64 * half, 64 * (half + 1))
                    for b in range(BPC):
                        col = 32 * b + h
                        nc.vector.tensor_copy(
                            out=qbd_f[rows, c, col : col + 1],
                            in_=qn_sb[rows, c, b : b + 1],
                        )
            qbd_bf = const.tile([128, CT, 32 * BPC], BF16)
            nc.vector.tensor_copy(out=qbd_bf, in_=qbd_f)
            # fold q into the K projection: raw = (Wk^T Qbd)^T x + Qbd^T bk
            for m in range(CT):
                wtp = kp_pool.tile([128, 32 * BPC], F32, tag="kp")
                for ot in range(CT):
                    nc.tensor.matmul(
                        wtp, wk2_sb[:, ot, ts(m, 128)], qbd_bf[:, ot, :],
                        start=(ot == 0), stop=(ot == CT - 1),
                    )
                nc.vector.tensor_copy(out=wtld_bf[:, m, :], in_=wtp)
            qbkp = kp_pool.tile([32 * BPC, 1], F32, tag="kp")
            for ot in range(CT):
                nc.tensor.matmul(
                    qbkp, qbd_f[:, ot, :], bk_sb[:, ot, None],
                    start=(ot == 0), stop=(ot == CT - 1),
                )
            nc.vector.tensor_copy(out=qbk_sb, in_=qbkp)

        # ---- per-batch pass A, split into k-part / score-part ---------------
        nmx8_b = [None] * BPC
        attnT_b = [None] * BPC
        rse_b = [None] * BPC
        xb_t = {}
        k2_t = {}
        pending_tp = {0: [], 1: []}

        def kpart(b, i, xbd, defer_tp=False):
            x_b = x_d[b].rearrange("(c p) n -> p c n", p=128)
            xf = xf_pool.tile([128, CT, FT], F32, name=f"xf{b}_{i}", tag="xf")
            nc.sync.dma_start(xf, x_b[:, :, ts(i, FT)])
            xb = xb_pool.tile([128, CT, FT], BF16, name=f"xb{b}_{i}", tag="xb")
            for c in range(CT):
                nc.vector.tensor_copy(out=xb[:, c, :], in_=xf[:, c, :])
            xb_t[(b, i)] = xb
            # bf16 bounce (gpsimd/SWDGE ring), 2 f-tiles per chunk
            ch, off = divmod(i * FT, CHW)
            nc.gpsimd.dma_start(
                xbd[ch].rearrange("(c p) n -> p c n", p=128)[:, :, off : off + FT],
                xb,
            )
            if off + FT == CHW:
                if defer_tp:
                    pending_tp[b].append(ch)
                else:
                    nc.sync.dma_start_transpose(
                        xT_all[:, ch * (CHW // 128) : (ch + 1) * (CHW // 128), :],
                        xbd[ch][:],
                    )
            k2sb = k2_pool.tile([128, CT, FT], BF16, name=f"k2_{b}_{i}", tag="k2")
            k2_t[(b, i)] = k2sb
            for o in range(CT):
                kp = kp_pool.tile([128, FT], F32, tag="kp")
                for c in range(CT):
                    nc.tensor.matmul(
                        kp, wk_sb[:, c, ts(o, 128)], xb[:, c, :],
                        start=(c == 0), stop=(c == CT - 1),
                    )
                nc.scalar.activation(
                    out=k2sb[:, o, :], in_=kp, func=AF.Square,
                    bias=bk_sb[:, o : o + 1], scale=1.0,
                )

        def flush_tp(b, xbd):
            for ch in pending_tp[b]:
                nc.sync.dma_start_transpose(
                    xT_all[:, ch * (CHW // 128) : (ch + 1) * (CHW // 128), :],
                    xbd[ch][:],
                )
            pending_tp[b] = []

        def spart(b, i):
            R = slice(32 * b, 32 * b + HEADS)
            xb = xb_t.pop((b, i))
            k2sb = k2_t.pop((b, i))
            sp = kp_pool.tile([32 * BPC, FT], F32, tag="kp")
            for c in range(CT):
                nc.tensor.matmul(
                    sp, wtld_bf[:, c, :], xb[:, c, :],
                    start=(c == 0), stop=(c == CT - 1),
                )
            sq = kp_pool.tile([HEADS, FT], F32, tag="kp")
            for c in range(CT):
                nc.tensor.matmul(
                    sq, ones_bf[:, c, :], k2sb[:, c, :],
                    start=(c == 0), stop=(c == CT - 1),
                )
            rt = small.tile([HEADS, FT], F32, tag="rt")
            nc.scalar.activation(out=rt, in_=sq, func=AF.Ln)
            nc.scalar.activation(out=rt, in_=rt, func=AF.Exp, scale=-0.5)
            nc.vector.tensor_scalar_min(rt, rt, 1.0 / EPS)
            nc.vector.tensor_scalar(
                out=sp[R, :], in0=sp[R, :],
                scalar1=qbk_sb[R], scalar2=None, op0=OP.add,
            )
            nc.vector.tensor_tensor(
                out=scores_all[R, ts(i, FT)], in0=sp[R, :], in1=rt, op=OP.mult,
            )
            nc.vector.tensor_reduce(
                nmx8_b[b][R, i : i + 1], scores_all[R, ts(i, FT)],
                axis=AX.X, op=OP.max)

        def softmax_attn(b):
            R = slice(32 * b, 32 * b + HEADS)
            nmx = small.tile([64, 1], F32, tag="st", name=f"nmx{b}")
            nc.vector.tensor_reduce(
                nmx[R], nmx8_b[b][R, :], axis=AX.X, op=OP.max, negate=True)
            rse = small.tile([64, 1], F32, tag="st", name=f"rse{b}")
            attnT_t = []
            se_t = []
            for chk in range(NCH):
                abt = at_pool.tile([64, CHW], BF16, tag="ab", name=f"ab{b}_{chk}")
                sec = small.tile([64, 1], F32, tag="se", name=f"se{b}_{chk}")
                nc.vector.memset(sec[R], 0.0)
                nc.scalar.activation(
                    out=abt[R, :], in_=scores_all[R, ts(chk, CHW)], func=AF.Exp,
                    bias=nmx[R], scale=1.0, accum_out=sec[R],
                )
                se_t.append(sec)
                att = at_pool.tile(
                    [128, CHW // 128, 32], BF16, tag="attnT", name=f"att{b}_{chk}")
                nc.sync.dma_start_transpose(att, abt[32 * b : 32 * b + 32, :])
                attnT_t.append(att)
            nc.vector.tensor_tensor(
                out=se_t[0][R], in0=se_t[0][R], in1=se_t[1][R], op=OP.add)
            nc.vector.tensor_tensor(
                out=se_t[2][R], in0=se_t[2][R], in1=se_t[3][R], op=OP.add)
            nc.vector.tensor_tensor(
                out=se_t[0][R], in0=se_t[0][R], in1=se_t[2][R], op=OP.add)
            nc.vector.reciprocal(rse[R], se_t[0][R])
            attnT_b[b] = attnT_t
            rse_b[b] = rse

        def pool(b):
            R = slice(32 * b, 32 * b + HEADS)
            attnT_t = attnT_b[b]
            pp0 = pp_pool.tile([HEADS, 384], F32, tag="pp")
            pp1 = pp_pool.tile([HEADS, 384], F32, tag="pp")
            for nt in range(NNT):
                atl = attnT_t[nt // (CHW // 128)][:, nt % (CHW // 128), 0:HEADS]
                nc.tensor.matmul(
                    pp0, atl, xT_all[:, nt, 0:384],
                    start=(nt == 0), stop=(nt == NNT - 1), skip_group_check=True,
                )
                nc.tensor.matmul(
                    pp1, atl, xT_all[:, nt, 384:768],
                    start=(nt == 0), stop=(nt == NNT - 1), skip_group_check=True,
                )
            nc.vector.tensor_scalar_mul(pooledT_all[R, 0:384], pp0, rse_b[b][R])
            nc.vector.tensor_scalar_mul(pooledT_all[R, 384:768], pp1, rse_b[b][R])

        xbd_b = []
        for b in range(BPC):
            nmx8_b[b] = small.tile([64, NFT], F32, tag="nmx8", name=f"nmx8_{b}")
            xbd_b.append([
                dram.tile([C, CHW], BF16, tag="xbd", name=f"xbd{b}_{t}")
                for t in range(NCH)
            ])

        # ---- schedule -------------------------------------------------------
        kpart(0, 0, xbd_b[0])
        kpart(0, 1, xbd_b[0])
        qpath()
        for i in range(NFT):
            if i + 2 < NFT:
                kpart(0, i + 2, xbd_b[0])
            spart(0, i)
        softmax_attn(0)
        kpart(1, 0, xbd_b[1], defer_tp=True)
        kpart(1, 1, xbd_b[1], defer_tp=True)
        pool(0)
        flush_tp(1, xbd_b[1])
        for i in range(NFT):
            if i + 2 < NFT:
                kpart(1, i + 2, xbd_b[1])
            spart(1, i)
        softmax_attn(1)
        pool(1)

        # ---- tail: out = Wp @ (Wv @ pooled)|diag + bpz ---------------------
        wv_sb = const.tile([128, CT, C], BF16)
        nc.scalar.dma_start(wv_sb, wv_d.rearrange("(c p) o -> p c o", p=128))
        wp_sb = const.tile([128, CT, C], BF16)
        nc.scalar.dma_start(wp_sb, wp_d.rearrange("(c p) o -> p c o", p=128))
        pooled_sb = const.tile([128, CT, BPC * HEADS], BF16)
        for c in range(CT):
            tpp = kp_pool.tile([128, 64], F32, tag="kp")
            nc.tensor.transpose(tpp, pooledT_all[:, ts(c, 128)], id64_f)
            for b in range(BPC):
                nc.vector.tensor_copy(
                    out=pooled_sb[:, c, b * HEADS : (b + 1) * HEADS],
                    in_=tpp[:, 32 * b : 32 * b + HEADS])

        outv_sb = const.tile([128, CT, BPC], BF16)
        for o in range(CT):
            vp = kp_pool.tile([128, BPC * HEADS], F32, tag="kp")
            for c in range(CT):
                nc.tensor.matmul(
                    vp, wv_sb[:, c, ts(o, 128)], pooled_sb[:, c, :],
                    start=(c == 0), stop=(c == CT - 1),
                )
            for half in range(2):
                h = 2 * o + half
                rows = slice(64 * half, 64 * (half + 1))
                for b in range(BPC):
                    col = b * HEADS + h
                    nc.vector.tensor_copy(
                        out=outv_sb[rows, o, b : b + 1],
                        in_=vp[rows, col : col + 1],
                    )

        z_sb = const.tile([128, CT, BPC], F32)
        for o2 in range(CT):
            zp = kp_pool.tile([128, BPC], F32, tag="kp")
            for o in range(CT):
                nc.tensor.matmul(
                    zp, wp_sb[:, o, ts(o2, 128)], outv_sb[:, o, :],
                    start=(o == 0), stop=(o == CT - 1),
                )
            nc.vector.tensor_tensor(
                out=z_sb[:, o2, :], in0=zp,
                in1=bpz_sb[:, o2, None].to_broadcast((128, BPC)), op=OP.add,
            )
        nc.sync.dma_start(z_d.rearrange("(c p) b -> p c b", p=128), z_sb)


_NC_CACHE = None


def _get_nc():
    global _NC_CACHE
    if _NC_CACHE is None:
        _NC_CACHE = _build_nc()
    return _NC_CACHE


def make_in_maps(inputs):
    x = np.ascontiguousarray(np.asarray(inputs["x"], dtype=np.float32)).reshape(B, C, N)
    y = np.asarray(inputs["y"], dtype=np.float32).reshape(B, C)
    Wq = np.asarray(inputs["Wq"], dtype=np.float32)
    bq = np.asarray(inputs["bq"], dtype=np.float32)
    Wkv = np.asarray(inputs["Wkv"], dtype=np.float32)
    bkv = np.asarray(inputs["bkv"], dtype=np.float32)
    Wp = np.asarray(inputs["Wp"], dtype=np.float32)
    bp = np.asarray(inputs["bp"], dtype=np.float32)

    wk, wv = Wkv[:C], Wkv[C:]
    bk, bv = bkv[:C], bkv[C:]
    wkT = np.ascontiguousarray(wk.T).astype(ml_dtypes.bfloat16)
    wk2 = np.ascontiguousarray(wk).astype(ml_dtypes.bfloat16)
    wqT = np.ascontiguousarray(Wq.T).astype(ml_dtypes.bfloat16)
    wvT = np.ascontiguousarray(wv.T).astype(ml_dtypes.bfloat16)
    wpT = np.ascontiguousarray(Wp.T).astype(ml_dtypes.bfloat16)
    bpz = (Wp @ bv + bp).astype(np.float32)

    in_maps = []
    for i in range(NCORES):
        in_maps.append({
            "x": np.ascontiguousarray(x[i * BPC : (i + 1) * BPC]),
            "y": np.ascontiguousarray(y[i * BPC : (i + 1) * BPC].T),
            "wkT": wkT, "wk2": wk2, "wqT": wqT, "wvT": wvT, "wpT": wpT,
            "bq": bq, "bk": np.ascontiguousarray(bk),
            "bpz": bpz,
        })
    return in_maps


def kernel(**inputs):
    nc = _get_nc()
    in_maps = make_in_maps(inputs)
    res = run_bass_kernel_spmd(nc, in_maps, core_ids=list(range(NCORES)))
    z = np.concatenate([r["z"].T for r in res.results], axis=0)
    return z.reshape(B, C, 1, 1).astype(np.float32)


# revision 25
# speedup vs baseline: 1.1606x; 1.0226x over previous
"""Trainium2 Bass kernel for nn_C_Cross_Attention3D (B=16, C=768, H=W=64, HEADS=12).

Math (per batch b):
  q   = l2norm_per_head(Wq @ y_b + bq)                      # [12, 64]
  k   = Wk @ x_b + bk                                       # [768, N], N = 4096
  s   = (Qbd^T k) / max(||k||_head, eps)                    # [12, N] cosine scores
  a   = softmax_N(s)                                        # [12, N]
  out = Wp @ (Wv @ (x_b @ a^T |head-diag) + bv) + bp        # [768]

Key restructuring vs. the reference: the V projection commutes with the
attention pooling (one query token per head), so instead of projecting all
N tokens through Wv we pool x with the attention weights first:
  out_attn[head h] = Wv[h_rows, :] @ (x @ a_h^T)  + bv
This halves the dominant GEMM (only K projection runs over all tokens).

Transposes (x^T for the pooling contraction, a^T) are done by DMA-transpose
through a DRAM bounce buffer in bf16, keeping the PE free for matmuls.

Distribution: pure data-parallel over batch, 2 batches per core, 8 cores.
No collectives; host scatters inputs / gathers outputs.

Self-contained: hardcodes all shapes; no sibling imports.
"""

import numpy as np
import ml_dtypes

import concourse.bass as bass
import concourse.mybir as mybir
import concourse.tile as tile
from concourse import bacc
from concourse.bass import ts
from concourse.bass_utils import run_bass_kernel_spmd
from concourse.masks import make_identity

F32 = mybir.dt.float32
BF16 = mybir.dt.bfloat16
AF = mybir.ActivationFunctionType
OP = mybir.AluOpType
AX = mybir.AxisListType

B, C, HEADS, HD = 16, 768, 12, 64
N = 64 * 64                 # tokens per batch
NCORES = 8
BPC = B // NCORES           # batches per core = 2
CT = C // 128               # 6 c-tiles (contraction / channel tiles)
FT = 512                    # token f-tile size
NFT = N // FT               # 8 f-tiles
NNT = N // 128              # 32 n-tiles of 128 tokens
NCH = 4                     # x^T DMA-transpose chunks per batch
CHW = N // NCH              # chunk width in tokens (1024)
EPS = 1e-12


def _act_table_filter():
    """Restrict activation-table choice to the single set that covers all
    funcs this kernel uses (Copy/Exp/Ln/Square), so no mid-kernel
    ACT_TABLE_LOAD swaps are emitted. Index positions are preserved."""
    import functools
    import concourse.bacc as _bacc

    orig = _bacc.get_activation_tables

    @functools.cache
    def filtered(arch):
        t = orig(arch)
        return {
            name: (s if name == "natural_log_exp_and_others" else set())
            for name, s in t.items()
        }

    return orig, filtered


def _build_nc():
    nc = bacc.Bacc(
        "TRN2",
        target_bir_lowering=False,
        debug=False,
        enable_asserts=False,
        num_devices=NCORES,
    )

    x_d = nc.dram_tensor("x", [BPC, C, N], F32, kind="ExternalInput").ap()
    wk_d = nc.dram_tensor("wkT", [C, C], BF16, kind="ExternalInput").ap()
    wk2_d = nc.dram_tensor("wk2", [C, C], BF16, kind="ExternalInput").ap()
    wq_d = nc.dram_tensor("wqT", [C, C], BF16, kind="ExternalInput").ap()
    wv_d = nc.dram_tensor("wvT", [C, C], BF16, kind="ExternalInput").ap()
    wp_d = nc.dram_tensor("wpT", [C, C], BF16, kind="ExternalInput").ap()
    aux_d = nc.dram_tensor("aux", [128, CT, 8], F32, kind="ExternalInput").ap()
    z_d = nc.dram_tensor("z", [C, BPC], F32, kind="ExternalOutput").ap()

    with tile.TileContext(nc) as tc:
        _emit(nc, tc, x_d, wk_d, wk2_d, wq_d, wv_d, wp_d, aux_d, z_d)
    import concourse.bacc as _bacc
    orig, filtered = _act_table_filter()
    _bacc.get_activation_tables = filtered
    try:
        nc.compile()
    finally:
        _bacc.get_activation_tables = orig
    return nc


def _emit(nc, tc, x_d, wk_d, wk2_d, wq_d, wv_d, wp_d, aux_d, z_d):
    from contextlib import ExitStack

    ctx = ExitStack()
    with ctx:
        const = ctx.enter_context(tc.tile_pool(name="const", bufs=1))
        statics = ctx.enter_context(tc.tile_pool(name="statics", bufs=1))
        xf_pool = ctx.enter_context(tc.tile_pool(name="xf", bufs=2))
        xb_pool = ctx.enter_context(tc.tile_pool(name="xb", bufs=3))
        k2_pool = ctx.enter_context(tc.tile_pool(name="k2", bufs=3))
        small = ctx.enter_context(tc.tile_pool(name="small", bufs=4))
        at_pool = ctx.enter_context(tc.tile_pool(name="at", bufs=5))
        dram = ctx.enter_context(tc.tile_pool(name="dram", bufs=6, space="DRAM"))
        kp_pool = ctx.enter_context(tc.tile_pool(name="kp", bufs=6, space="PSUM"))
        pp_pool = ctx.enter_context(tc.tile_pool(name="pp", bufs=2, space="PSUM"))

        # ---- weights on the scalar ring; x streams on sync/gpsimd ---------
        aux_sb = const.tile([128, CT, 8], F32)
        nc.scalar.dma_start(aux_sb, aux_d)
        wk_sb = const.tile([128, CT, C], BF16)
        nc.scalar.dma_start(wk_sb, wk_d.rearrange("(c p) o -> p c o", p=128))
        wq_sb = const.tile([128, CT, C], BF16)
        nc.scalar.dma_start(wq_sb, wq_d.rearrange("(c p) o -> p c o", p=128))
        wk2_sb = const.tile([128, CT, C], BF16)
        nc.scalar.dma_start(wk2_sb, wk2_d.rearrange("(o p) c -> p o c", p=128))
        bq_sb = aux_sb[:, :, 0]
        bk_sb = aux_sb[:, :, 1]
        bpz_sb = aux_sb[:, :, 2]
        y_sb = aux_sb[:, :, 4:6]

        id128_f = const.tile([128, 128], F32)
        make_identity(nc, id128_f)
        id64_f = const.tile([64, 64], F32)
        make_identity(nc, id64_f)

        # ones_bd[c, h] = 1 if c // 64 == h  (block-diagonal head indicator)
        ones_bf = const.tile([128, CT, HEADS], BF16)
        ones_f = const.tile([128, CT, HEADS], F32)
        onesT_f = const.tile([HEADS, C], F32)
        nc.vector.memset(ones_bf, 0.0)
        nc.vector.memset(ones_f, 0.0)
        for c in range(CT):
            for half in range(2):
                h = 2 * c + half
                rows = slice(64 * half, 64 * (half + 1))
                nc.vector.memset(ones_bf[rows, c, h : h + 1], 1.0)
                nc.vector.memset(ones_f[rows, c, h : h + 1], 1.0)

        # ---- statics --------------------------------------------------------
        scores_all = statics.tile([64, N], F32)   # rows: 32*b + h
        nc.vector.memset(scores_all, 0.0)
        xT_all = statics.tile([128, NNT, C], BF16)  # transposed x, current batch
        pooledT_all = statics.tile([64, C], F32)
        nc.vector.memset(pooledT_all, 0.0)

        wtld_bf = const.tile([128, CT, 32 * BPC], BF16)
        qbk_sb = const.tile([32 * BPC, 1], F32)

        def qpath():
            y_bf = const.tile([128, CT, BPC], BF16)
            nc.vector.tensor_copy(out=y_bf, in_=y_sb)
            for c in range(CT):
                otp = kp_pool.tile([HEADS, 128], F32, tag="kp")
                nc.tensor.transpose(otp, ones_f[:, c, :], id128_f)
                nc.scalar.copy(out=onesT_f[:, ts(c, 128)], in_=otp)
            q_sb = const.tile([128, CT, BPC], F32)
            for o in range(CT):
                qp = kp_pool.tile([128, BPC], F32, tag="kp")
                for c in range(CT):
                    nc.tensor.matmul(
                        qp, wq_sb[:, c, ts(o, 128)], y_bf[:, c, :],
                        start=(c == 0), stop=(c == CT - 1),
                    )
                nc.vector.tensor_tensor(
                    out=q_sb[:, o, :], in0=qp,
                    in1=aux_sb[:, o, 0:1].to_broadcast((128, BPC)), op=OP.add,
                )
            q2_sb = const.tile([128, CT, BPC], F32)
            nc.scalar.activation(out=q2_sb, in_=q_sb, func=AF.Square)
            ssqq = kp_pool.tile([HEADS, BPC], F32, tag="kp")
            for c in range(CT):
                nc.tensor.matmul(
                    ssqq, ones_f[:, c, :], q2_sb[:, c, :],
                    start=(c == 0), stop=(c == CT - 1),
                )
            rq = const.tile([HEADS, BPC], F32)
            nc.scalar.activation(out=rq, in_=ssqq, func=AF.Ln)
            nc.scalar.activation(out=rq, in_=rq, func=AF.Exp, scale=-0.5)
            nc.vector.tensor_scalar_min(rq, rq, 1.0 / EPS)
            rqbc = kp_pool.tile([128, CT, BPC], F32, tag="kp")
            for c in range(CT):
                nc.tensor.matmul(
                    rqbc[:, c, :], onesT_f[:, ts(c, 128)], rq,
                    start=(c == 0), stop=(c == CT - 1), skip_group_check=True,
                )
            qn_sb = const.tile([128, CT, BPC], F32)
            nc.vector.tensor_tensor(out=qn_sb, in0=q_sb, in1=rqbc, op=OP.mult)
            qbd_f = const.tile([128, CT, 32 * BPC], F32)
            nc.vector.memset(qbd_f, 0.0)
            for c in range(CT):
                for half in range(2):
                    h = 2 * c + half
                    rows = slice(64 * half, 64 * (half + 1))
                    for b in range(BPC):
                        col = 32 * b + h
                        nc.vector.tensor_copy(
                            out=qbd_f[rows, c, col : col + 1],
                            in_=qn_sb[rows, c, b : b + 1],
                        )
            qbd_bf = const.tile([128, CT, 32 * BPC], BF16)
            nc.vector.tensor_copy(out=qbd_bf, in_=qbd_f)
            # fold q into the K projection: raw = (Wk^T Qbd)^T x + Qbd^T bk
            for m in range(CT):
                wtp = kp_pool.tile([128, 32 * BPC], F32, tag="kp")
                for ot in range(CT):
                    nc.tensor.matmul(
                        wtp, wk2_sb[:, ot, ts(m, 128)], qbd_bf[:, ot, :],
                        start=(ot == 0), stop=(ot == CT - 1),
                    )
                nc.vector.tensor_copy(out=wtld_bf[:, m, :], in_=wtp)
            qbkp = kp_pool.tile([32 * BPC, 1], F32, tag="kp")
            for ot in range(CT):
                nc.tensor.matmul(
                    qbkp, qbd_f[:, ot, :], aux_sb[:, ot, 1:2],
                    start=(ot == 0), stop=(ot == CT - 1),
                )
            nc.vector.tensor_copy(out=qbk_sb, in_=qbkp)

        # ---- per-batch pass A, split into k-part / score-part ---------------
        nmx8_b = [None] * BPC
        attnT_b = [None] * BPC
        rse_b = [None] * BPC
        xb_t = {}
        k2_t = {}
        pending_tp = {0: [], 1: []}

        def kpart(b, i, xbd, defer_tp=False):
            x_b = x_d[b].rearrange("(c p) n -> p c n", p=128)
            xf = xf_pool.tile([128, CT, FT], F32, name=f"xf{b}_{i}", tag="xf")
            nc.sync.dma_start(xf[:, 0:3, :], x_b[:, 0:3, ts(i, FT)])
            nc.gpsimd.dma_start(xf[:, 3:6, :], x_b[:, 3:6, ts(i, FT)])
            xb = xb_pool.tile([128, CT, FT], BF16, name=f"xb{b}_{i}", tag="xb")
            for c in range(CT):
                nc.vector.tensor_copy(out=xb[:, c, :], in_=xf[:, c, :])
            xb_t[(b, i)] = xb
            # bf16 bounce (gpsimd/SWDGE ring), 2 f-tiles per chunk
            ch, off = divmod(i * FT, CHW)
            nc.gpsimd.dma_start(
                xbd[ch].rearrange("(c p) n -> p c n", p=128)[:, :, off : off + FT],
                xb,
            )
            if off + FT == CHW:
                if defer_tp:
                    pending_tp[b].append(ch)
                else:
                    nc.sync.dma_start_transpose(
                        xT_all[:, ch * (CHW // 128) : (ch + 1) * (CHW // 128), :],
                        xbd[ch][:],
                    )
            k2sb = k2_pool.tile([128, CT, FT], BF16, name=f"k2_{b}_{i}", tag="k2")
            k2_t[(b, i)] = k2sb
            for o in range(CT):
                kp = kp_pool.tile([128, FT], F32, tag="kp")
                for c in range(CT):
                    nc.tensor.matmul(
                        kp, wk_sb[:, c, ts(o, 128)], xb[:, c, :],
                        start=(c == 0), stop=(c == CT - 1),
                    )
                nc.scalar.activation(
                    out=k2sb[:, o, :], in_=kp, func=AF.Square,
                    bias=aux_sb[:, o, 1:2], scale=1.0,
                )

        def flush_tp(b, xbd):
            for ch in pending_tp[b]:
                nc.sync.dma_start_transpose(
                    xT_all[:, ch * (CHW // 128) : (ch + 1) * (CHW // 128), :],
                    xbd[ch][:],
                )
            pending_tp[b] = []

        def spart(b, i):
            R = slice(32 * b, 32 * b + HEADS)
            xb = xb_t.pop((b, i))
            k2sb = k2_t.pop((b, i))
            sp = kp_pool.tile([32 * BPC, FT], F32, tag="kp")
            for c in range(CT):
                nc.tensor.matmul(
                    sp, wtld_bf[:, c, :], xb[:, c, :],
                    start=(c == 0), stop=(c == CT - 1),
                )
            sq = kp_pool.tile([HEADS, FT], F32, tag="kp")
            for c in range(CT):
                nc.tensor.matmul(
                    sq, ones_bf[:, c, :], k2sb[:, c, :],
                    start=(c == 0), stop=(c == CT - 1),
                )
            rt = small.tile([HEADS, FT], F32, tag="rt")
            nc.scalar.activation(out=rt, in_=sq, func=AF.Ln)
            nc.scalar.activation(out=rt, in_=rt, func=AF.Exp, scale=-0.5)
            nc.vector.tensor_scalar_min(rt, rt, 1.0 / EPS)
            nc.vector.tensor_scalar(
                out=sp[R, :], in0=sp[R, :],
                scalar1=qbk_sb[R], scalar2=None, op0=OP.add,
            )
            nc.vector.tensor_tensor(
                out=scores_all[R, ts(i, FT)], in0=sp[R, :], in1=rt, op=OP.mult,
            )
            nc.vector.tensor_reduce(
                nmx8_b[b][R, i : i + 1], scores_all[R, ts(i, FT)],
                axis=AX.X, op=OP.max)

        def softmax_attn(b):
            R = slice(32 * b, 32 * b + HEADS)
            nmx = small.tile([64, 1], F32, tag="st", name=f"nmx{b}")
            nc.vector.tensor_reduce(
                nmx[R], nmx8_b[b][R, :], axis=AX.X, op=OP.max, negate=True)
            rse = small.tile([64, 1], F32, tag="st", name=f"rse{b}")
            attnT_t = []
            se_t = []
            for chk in range(NCH):
                abt = at_pool.tile([64, CHW], BF16, tag="ab", name=f"ab{b}_{chk}")
                sec = small.tile([64, 1], F32, tag="se", name=f"se{b}_{chk}")
                nc.vector.memset(sec[R], 0.0)
                nc.scalar.activation(
                    out=abt[R, :], in_=scores_all[R, ts(chk, CHW)], func=AF.Exp,
                    bias=nmx[R], scale=1.0, accum_out=sec[R],
                )
                se_t.append(sec)
                att = at_pool.tile(
                    [128, CHW // 128, 32], BF16, tag="attnT", name=f"att{b}_{chk}")
                nc.sync.dma_start_transpose(att, abt[32 * b : 32 * b + 32, :])
                attnT_t.append(att)
            nc.vector.tensor_tensor(
                out=se_t[0][R], in0=se_t[0][R], in1=se_t[1][R], op=OP.add)
            nc.vector.tensor_tensor(
                out=se_t[2][R], in0=se_t[2][R], in1=se_t[3][R], op=OP.add)
            nc.vector.tensor_tensor(
                out=se_t[0][R], in0=se_t[0][R], in1=se_t[2][R], op=OP.add)
            nc.vector.reciprocal(rse[R], se_t[0][R])
            attnT_b[b] = attnT_t
            rse_b[b] = rse

        def pool(b):
            R = slice(32 * b, 32 * b + HEADS)
            attnT_t = attnT_b[b]
            pp0 = pp_pool.tile([HEADS, 384], F32, tag="pp")
            pp1 = pp_pool.tile([HEADS, 384], F32, tag="pp")
            for nt in range(NNT):
                atl = attnT_t[nt // (CHW // 128)][:, nt % (CHW // 128), 0:HEADS]
                nc.tensor.matmul(
                    pp0, atl, xT_all[:, nt, 0:384],
                    start=(nt == 0), stop=(nt == NNT - 1), skip_group_check=True,
                )
                nc.tensor.matmul(
                    pp1, atl, xT_all[:, nt, 384:768],
                    start=(nt == 0), stop=(nt == NNT - 1), skip_group_check=True,
                )
            nc.vector.tensor_scalar_mul(pooledT_all[R, 0:384], pp0, rse_b[b][R])
            nc.vector.tensor_scalar_mul(pooledT_all[R, 384:768], pp1, rse_b[b][R])

        xbd_b = []
        for b in range(BPC):
            nmx8_b[b] = small.tile([64, NFT], F32, tag="nmx8", name=f"nmx8_{b}")
            xbd_b.append([
                dram.tile([C, CHW], BF16, tag="xbd", name=f"xbd{b}_{t}")
                for t in range(NCH)
            ])

        # ---- schedule -------------------------------------------------------
        kpart(0, 0, xbd_b[0])
        kpart(0, 1, xbd_b[0])
        qpath()
        for i in range(NFT):
            if i + 2 < NFT:
                kpart(0, i + 2, xbd_b[0])
            spart(0, i)
        softmax_attn(0)
        kpart(1, 0, xbd_b[1], defer_tp=True)
        kpart(1, 1, xbd_b[1], defer_tp=True)
        pool(0)
        flush_tp(1, xbd_b[1])
        for i in range(NFT):
            if i + 2 < NFT:
                kpart(1, i + 2, xbd_b[1])
            spart(1, i)
        softmax_attn(1)
        pool(1)

        # ---- tail: out = Wp @ (Wv @ pooled)|diag + bpz ---------------------
        wv_sb = const.tile([128, CT, C], BF16)
        nc.scalar.dma_start(wv_sb, wv_d.rearrange("(c p) o -> p c o", p=128))
        wp_sb = const.tile([128, CT, C], BF16)
        nc.scalar.dma_start(wp_sb, wp_d.rearrange("(c p) o -> p c o", p=128))
        pooled_sb = const.tile([128, CT, BPC * HEADS], BF16)
        for c in range(CT):
            tpp = kp_pool.tile([128, 64], F32, tag="kp")
            nc.tensor.transpose(tpp, pooledT_all[:, ts(c, 128)], id64_f)
            for b in range(BPC):
                nc.vector.tensor_copy(
                    out=pooled_sb[:, c, b * HEADS : (b + 1) * HEADS],
                    in_=tpp[:, 32 * b : 32 * b + HEADS])

        outv_sb = const.tile([128, CT, BPC], BF16)
        for o in range(CT):
            vp = kp_pool.tile([128, BPC * HEADS], F32, tag="kp")
            for c in range(CT):
                nc.tensor.matmul(
                    vp, wv_sb[:, c, ts(o, 128)], pooled_sb[:, c, :],
                    start=(c == 0), stop=(c == CT - 1),
                )
            for half in range(2):
                h = 2 * o + half
                rows = slice(64 * half, 64 * (half + 1))
                for b in range(BPC):
                    col = b * HEADS + h
                    nc.vector.tensor_copy(
                        out=outv_sb[rows, o, b : b + 1],
                        in_=vp[rows, col : col + 1],
                    )

        z_sb = const.tile([128, CT, BPC], F32)
        for o2 in range(CT):
            zp = kp_pool.tile([128, BPC], F32, tag="kp")
            for o in range(CT):
                nc.tensor.matmul(
                    zp, wp_sb[:, o, ts(o2, 128)], outv_sb[:, o, :],
                    start=(o == 0), stop=(o == CT - 1),
                )
            nc.vector.tensor_tensor(
                out=z_sb[:, o2, :], in0=zp,
                in1=aux_sb[:, o2, 2:3].to_broadcast((128, BPC)), op=OP.add,
            )
        nc.sync.dma_start(z_d.rearrange("(c p) b -> p c b", p=128), z_sb)


_NC_CACHE = None


def _get_nc():
    global _NC_CACHE
    if _NC_CACHE is None:
        _NC_CACHE = _build_nc()
    return _NC_CACHE


def make_in_maps(inputs):
    x = np.ascontiguousarray(np.asarray(inputs["x"], dtype=np.float32)).reshape(B, C, N)
    y = np.asarray(inputs["y"], dtype=np.float32).reshape(B, C)
    Wq = np.asarray(inputs["Wq"], dtype=np.float32)
    bq = np.asarray(inputs["bq"], dtype=np.float32)
    Wkv = np.asarray(inputs["Wkv"], dtype=np.float32)
    bkv = np.asarray(inputs["bkv"], dtype=np.float32)
    Wp = np.asarray(inputs["Wp"], dtype=np.float32)
    bp = np.asarray(inputs["bp"], dtype=np.float32)

    wk, wv = Wkv[:C], Wkv[C:]
    bk, bv = bkv[:C], bkv[C:]
    wkT = np.ascontiguousarray(wk.T).astype(ml_dtypes.bfloat16)
    wk2 = np.ascontiguousarray(wk).astype(ml_dtypes.bfloat16)
    wqT = np.ascontiguousarray(Wq.T).astype(ml_dtypes.bfloat16)
    wvT = np.ascontiguousarray(wv.T).astype(ml_dtypes.bfloat16)
    wpT = np.ascontiguousarray(Wp.T).astype(ml_dtypes.bfloat16)
    bpz = (Wp @ bv + bp).astype(np.float32)

    def pcol(v):
        return v.reshape(CT, 128).T  # [(c p)] -> [p, c]

    in_maps = []
    for i in range(NCORES):
        aux = np.zeros((128, CT, 8), np.float32)
        aux[:, :, 0] = pcol(bq)
        aux[:, :, 1] = pcol(bk)
        aux[:, :, 2] = pcol(bpz)
        yb = y[i * BPC : (i + 1) * BPC]  # [2, C]
        for b in range(BPC):
            aux[:, :, 4 + b] = pcol(yb[b])
        in_maps.append({
            "x": np.ascontiguousarray(x[i * BPC : (i + 1) * BPC]),
            "wkT": wkT, "wk2": wk2, "wqT": wqT, "wvT": wvT, "wpT": wpT,
            "aux": aux,
        })
    return in_maps


def kernel(**inputs):
    nc = _get_nc()
    in_maps = make_in_maps(inputs)
    res = run_bass_kernel_spmd(nc, in_maps, core_ids=list(range(NCORES)))
    z = np.concatenate([r["z"].T for r in res.results], axis=0)
    return z.reshape(B, C, 1, 1).astype(np.float32)
